# revision 14
# baseline (speedup 1.0000x reference)
"""Trainium2 Bass kernel for a dense transformer decoder block on 8 NeuronCores.

Sharding (uniform SPMD):
  * tokens: core c owns 512 contiguous tokens — batch c//4, positions
    [512*(c%4), 512*(c%4)+512). All projections, norms and the FFN are
    computed purely locally on those tokens.
  * attention: head-parallel via AllToAll, pipelined in two per-head rounds.
    Each core computes Q/K/V for its own tokens (all heads, feature-major,
    RoPE applied to Q/K). Round r redistributes head 2c+r: two kv AllToAlls
    (round a kicked while round b is bounced, both overlap the Q projection),
    one q AllToAll, and per-round o AllToAlls that overlap the other round's
    attention compute. Causal attention for the round's head runs fully
    on-chip; the o AllToAll routes outputs back to token owners where the
    output projection accumulates the two rounds (partial sums in SBUF).

Layout: activations are feature-major (features on SBUF partitions, tokens on
the free axis) so every matmul is transpose-free. Scores are built in S^T
orientation (kv on partitions) feeding softmax (exp on ScalarE, key-padding
mask folded into the exp bias, causal diagonal via an additive band mask)
straight into the attention*V matmul; the softmax denominator is a
ones-column matmul accumulated alongside. RMSNorm statistics use a
Square-activation + ones-matmul (cross-partition reduce on the PE).

Dtypes: bf16 operands for ALL matmuls (PSUM accumulation stays fp32), which
halves weight/A2A/DVE traffic vs fp32r at the same PE rate. The residual
path (x, x+attn) stays fp32 SBUF-resident across phases — no DRAM spill.
Norm statistics run in fp32r.

Engine assignment: sync(SP)=weight streaming, scalar(ACT)=activations +
bounce/gather DMAs, gpsimd=collectives + broadcasts + output stores.
"""
import sys

sys.path.insert(0, '/opt/trn_rl_repo')

import numpy as np
import ml_dtypes

import concourse.bacc as bacc
import concourse.mybir as mybir
from concourse import tile
from concourse.bass_utils import run_bass_kernel_spmd

F32 = mybir.dt.float32
F32R = mybir.dt.float32r
BF16 = mybir.dt.bfloat16
AF = mybir.ActivationFunctionType

D = 2048
H = 16
DH = 128
FF = 8192
B = 2
L = 2048
NCORES = 8
TOK = 512            # tokens per core
NF = D // 128        # 16 feature tiles
NEG = -30000.0
EPS = float(np.finfo(np.float32).eps)
ISQ = 1.0 / float(np.sqrt(DH))
RG = [list(range(NCORES))]


def _build():
    nc = bacc.Bacc("TRN2", target_bir_lowering=False, debug=False,
                   num_devices=NCORES)

    xT = nc.dram_tensor("xT", [D, TOK], F32, kind="ExternalInput")
    wq = nc.dram_tensor("wq", [D, D], BF16, kind="ExternalInput")
    wk = nc.dram_tensor("wk", [D, D], BF16, kind="ExternalInput")
    wv = nc.dram_tensor("wv", [D, D], BF16, kind="ExternalInput")
    wo = nc.dram_tensor("wo", [D, D], BF16, kind="ExternalInput")
    wf1 = nc.dram_tensor("wf1", [D, FF], BF16, kind="ExternalInput")
    wf2 = nc.dram_tensor("wf2", [FF, D], BF16, kind="ExternalInput")
    ropeC = nc.dram_tensor("ropeC", [DH, TOK], F32, kind="ExternalInput")
    ropeS2 = nc.dram_tensor("ropeS2", [DH, TOK], F32, kind="ExternalInput")
    band = nc.dram_tensor("band", [128, 896], BF16, kind="ExternalInput")
    mbias = nc.dram_tensor("mbias", [128, 2 * H], F32, kind="ExternalInput")
    onesr = nc.dram_tensor("onesr", [128, 1], F32R, kind="ExternalInput")
    onesb = nc.dram_tensor("onesb", [128, 1], BF16, kind="ExternalInput")
    outT = nc.dram_tensor("outT", [D, TOK], F32, kind="ExternalOutput")

    # internal DRAM AllToAll bounce buffers (bf16)
    kvinA = nc.dram_tensor("kvinA", [NCORES * 256, TOK], BF16)
    kvinB = nc.dram_tensor("kvinB", [NCORES * 256, TOK], BF16)
    kvoutA = nc.dram_tensor("kvoutA", [NCORES * 256, TOK], BF16)
    kvoutB = nc.dram_tensor("kvoutB", [NCORES * 256, TOK], BF16)
    qinA = nc.dram_tensor("qinA", [NCORES * 128, TOK], BF16)
    qinB = nc.dram_tensor("qinB", [NCORES * 128, TOK], BF16)
    qoutA = nc.dram_tensor("qoutA", [NCORES * 128, TOK], BF16)
    qoutB = nc.dram_tensor("qoutB", [NCORES * 128, TOK], BF16)
    oinA = nc.dram_tensor("oinA", [NCORES * 128, TOK], BF16)
    oinB = nc.dram_tensor("oinB", [NCORES * 128, TOK], BF16)
    ooutA = nc.dram_tensor("ooutA", [NCORES * 128, TOK], BF16)
    ooutB = nc.dram_tensor("ooutB", [NCORES * 128, TOK], BF16)

    with tile.TileContext(nc) as tc:
        with (
            tc.tile_pool(name="const", bufs=1) as cp,
            tc.tile_pool(name="small", bufs=1) as sp,
        ):
            cosT = cp.tile([DH, TOK], F32)
            sin2 = cp.tile([DH, TOK], F32)
            bandT = cp.tile([128, 896], BF16)
            mbT = cp.tile([128, 2 * H], F32)
            onr = cp.tile([128, 1], F32R)
            onb = cp.tile([128, 1], BF16)
            epsc = cp.tile([1, 1], F32)
            x2 = cp.tile([128, NF * TOK], F32)      # resident x + attn@wo
            nc.scalar.dma_start(onr[:], onesr[:])
            nc.gpsimd.memset(epsc[:], EPS)

            rsB = sp.tile([128, TOK], F32)
            rowS = sp.tile([1, TOK], F32)
            rowR = sp.tile([1, TOK], F32)

            def rmsnorm_rs(ssq_ps):
                nc.scalar.activation(rowS[:], ssq_ps[:], AF.Sqrt,
                                     bias=epsc[:], scale=1.0 / D)
                nc.vector.reciprocal(rowR[:], rowS[:])
                nc.gpsimd.partition_broadcast(rsB[:], rowR[:])

            with tc.tile_pool(name="mx", bufs=1) as mxp:
                xt = mxp.tile([128, NF * TOK], F32)   # resident input x^T
                for qq in range(4):
                    nc.scalar.dma_start(
                        xt[:, qq * 4 * TOK:(qq + 1) * 4 * TOK]
                        .rearrange("p (i c) -> p i c", i=4),
                        xT[qq * 512:(qq + 1) * 512, :]
                        .rearrange("(i p) c -> p i c", p=128))
                nc.scalar.dma_start(cosT[:], ropeC[:])
                nc.scalar.dma_start(sin2[:], ropeS2[:])
                nc.scalar.dma_start(bandT[:], band[:])
                nc.scalar.dma_start(mbT[:], mbias[:])
                nc.scalar.dma_start(onb[:], onesb[:])

                # ========== Phase 1: norm1, K^T, V, Q^T, AllToAlls ==========
                with (
                    tc.tile_pool(name="m1", bufs=1) as m1,
                    tc.tile_pool(name="ps1", bufs=3, space="PSUM") as ps1,
                    tc.tile_pool(name="psr", bufs=1, space="PSUM") as psr,
                ):
                    ssq = psr.tile([1, TOK], F32, tag="row")
                    for i in range(NF):
                        sq = sp.tile([128, TOK], F32R, tag="sq", bufs=2)
                        nc.scalar.activation(sq[:],
                                             xt[:, i * TOK:(i + 1) * TOK],
                                             AF.Square)
                        nc.tensor.matmul(ssq[:], onr[:], sq[:],
                                         start=(i == 0), stop=(i == NF - 1))
                    rmsnorm_rs(ssq)
                    xnt = m1.tile([128, NF * TOK], BF16, tag="xn")
                    for i in range(NF):
                        nc.vector.tensor_mul(xnt[:, i * TOK:(i + 1) * TOK],
                                             xt[:, i * TOK:(i + 1) * TOK],
                                             rsB[:])

                    def proj_T(wten, out_tile, tag, o_range):
                        """out_tile[:, o*TOK:] = head-tile o of (xn @ w)^T,
                        roped."""
                        for o in o_range:
                            wc = m1.tile([128, NF * 128], BF16, tag=tag,
                                         bufs=3)
                            nc.sync.dma_start(
                                wc[:].rearrange("p (i m) -> p i m", i=NF),
                                wten[:, o * 128:(o + 1) * 128]
                                .rearrange("(i p) m -> p i m", p=128))
                            acc = ps1.tile([128, TOK], F32, tag="big")
                            for i in range(NF):
                                nc.tensor.matmul(
                                    acc[:], wc[:, i * 128:(i + 1) * 128],
                                    xnt[:, i * TOK:(i + 1) * TOK],
                                    start=(i == 0), stop=(i == NF - 1))
                            dst = out_tile[:, o * TOK:(o + 1) * TOK]
                            tmp = sp.tile([128, TOK], BF16, tag="rtmp",
                                          bufs=2)
                            nc.vector.tensor_mul(tmp[0:64, :], acc[64:128, :],
                                                 sin2[0:64, :])
                            nc.vector.tensor_mul(tmp[64:128, :], acc[0:64, :],
                                                 sin2[64:128, :])
                            nc.vector.tensor_mul(dst, acc[:], cosT[:])
                            nc.vector.tensor_add(dst, dst, tmp[:])

                    kt = m1.tile([128, NF * TOK], BF16, tag="kt")
                    vt = m1.tile([128, 4 * D], BF16, tag="vt")
                    qt = m1.tile([128, NF * TOK], BF16, tag="qt")
                    ktv = kt[:].rearrange("p (o c) -> p o c", o=NF)
                    vtv = vt[:].rearrange("p (t f) -> p t f", t=4)
                    qtv = qt[:].rearrange("p (o c) -> p o c", o=NF)

                    def v_half(cc_range):
                        for cc in cc_range:
                            wvc = m1.tile([128, NF * TOK], BF16, tag="wvc",
                                          bufs=2, name="wvc")
                            nc.sync.dma_start(
                                wvc[:].rearrange("p (i m) -> p i m", i=NF),
                                wv[:, cc * TOK:(cc + 1) * TOK]
                                .rearrange("(i p) m -> p i m", p=128))
                            for to in range(4):
                                acc = ps1.tile([128, TOK], F32, tag="big",
                                               name="acc")
                                for i in range(NF):
                                    nc.tensor.matmul(
                                        acc[:],
                                        xnt[:, i * TOK + to * 128:
                                            i * TOK + (to + 1) * 128],
                                        wvc[:, i * TOK:(i + 1) * TOK],
                                        start=(i == 0), stop=(i == NF - 1))
                                nc.vector.tensor_copy(
                                    vt[:, to * D + cc * TOK:
                                       to * D + (cc + 1) * TOK], acc[:])

                    def bounce_kv(r, kvin_t, kvout_t):
                        # round r carries head j+8r for dest core j
                        kviv = kvin_t.ap().rearrange(
                            "(j s d) (t f) -> j s d t f", j=NCORES, s=2,
                            d=128, t=4, f=128)
                        for j in range(NCORES):
                            hh = j + 8 * r
                            nc.scalar.dma_start(
                                kviv[j, 0].rearrange("d t f -> d (t f)"),
                                ktv[:, hh, :])
                            nc.scalar.dma_start(
                                kviv[j, 1],
                                vtv[:, :, hh * 128:(hh + 1) * 128])
                        nc.gpsimd.collective_compute(
                            "AllToAll", mybir.AluOpType.bypass,
                            replica_groups=RG,
                            ins=[kvin_t.ap().opt()],
                            outs=[kvout_t.ap().opt()])

                    def bounce_q(r, qin_t, qout_t):
                        qiv = qin_t.ap().rearrange("(j d) c -> j d c",
                                                   j=NCORES, d=128)
                        for j in range(NCORES):
                            nc.scalar.dma_start(qiv[j], qtv[:, j + 8 * r, :])
                        nc.gpsimd.collective_compute(
                            "AllToAll", mybir.AluOpType.bypass,
                            replica_groups=RG,
                            ins=[qin_t.ap().opt()], outs=[qout_t.ap().opt()])

                    # 4-deep pipeline: each half ships while the next computes
                    proj_T(wk, kt, "wc", range(0, 8))
                    v_half(range(0, 2))
                    bounce_kv(0, kvinA, kvoutA)
                    proj_T(wk, kt, "wc", range(8, 16))
                    v_half(range(2, 4))
                    bounce_kv(1, kvinB, kvoutB)
                    proj_T(wq, qt, "wc", range(0, 8))
                    bounce_q(0, qinA, qoutA)
                    proj_T(wq, qt, "wc", range(8, 16))
                    bq1 = lambda: bounce_q(1, qinB, qoutB)

                # ====== Phase 2+3: attention rounds + O-projection ======
                with (
                    tc.tile_pool(name="m2", bufs=1) as m2,
                    tc.tile_pool(name="ps_s", bufs=2, space="PSUM") as ps_s,
                    tc.tile_pool(name="ps_av", bufs=2, space="PSUM") as ps_av,
                    tc.tile_pool(name="ps_dn", bufs=2, space="PSUM") as ps_dn,
                    tc.tile_pool(name="ps3", bufs=2, space="PSUM") as ps3,
                ):
                    ksb = [m2.tile([128, 8 * TOK], BF16, tag=f"ksb{r}",
                                   name=f"ksb{r}") for r in range(2)]
                    vsb = [m2.tile([128, 8 * TOK], BF16, tag=f"vsb{r}",
                                   name=f"vsb{r}") for r in range(2)]
                    qsb = [m2.tile([128, 8 * TOK], BF16, tag=f"qsb{r}",
                                   name=f"qsb{r}") for r in range(2)]
                    osb = [m2.tile([128, 8 * TOK], BF16, tag=f"osb{r}",
                                   name=f"osb{r}") for r in range(2)]
                    ao = [m2.tile([128, 8 * TOK], BF16, tag=f"ao{r}",
                                  name=f"ao{r}") for r in range(2)]
                    oacc = m2.tile([128, NF * TOK], BF16, tag="oacc")

                    kvout_t = (kvoutA, kvoutB)
                    qout_t = (qoutA, qoutB)
                    oin_t = (oinA, oinB)
                    oout_t = (ooutA, ooutB)

                    def load_round(r):
                        eng = nc.scalar if r == 0 else nc.sync
                        kvv = kvout_t[r].ap().rearrange(
                            "(j s d) c -> j s d c", j=NCORES, s=2, d=128)
                        qov = qout_t[r].ap().rearrange(
                            "(j d) c -> j d c", j=NCORES, d=128)
                        for j in range(NCORES):
                            eng.dma_start(
                                ksb[r][:, j * TOK:(j + 1) * TOK], kvv[j, 0])
                            eng.dma_start(
                                vsb[r][:, j * TOK:(j + 1) * TOK], kvv[j, 1])
                            eng.dma_start(
                                qsb[r][:, j * TOK:(j + 1) * TOK], qov[j])

                    def attn_round(r, cbs={}):
                        for b in range(B):
                            for q4 in range(4):
                                base = b * 4 * TOK
                                qs = qsb[r][:, base + q4 * TOK:
                                            base + (q4 + 1) * TOK]
                                ng = 4 * q4 + 4
                                av = ps_av.tile([128, TOK], F32, tag="av")
                                dn = ps_dn.tile([1, TOK], F32, tag="dn")
                                for g in range(ng):
                                    st = ps_s.tile([128, TOK], F32, tag="s")
                                    nc.tensor.matmul(
                                        st[:],
                                        ksb[r][:, base + g * 128:
                                               base + (g + 1) * 128],
                                        qs, start=True, stop=True)
                                    pt = sp.tile([128, TOK], BF16, tag="pt",
                                                 bufs=3)
                                    nc.scalar.activation(
                                        pt[:], st[:], AF.Exp,
                                        bias=mbT[:, b * H + g:b * H + g + 1],
                                        scale=ISQ)
                                    if g >= 4 * q4:
                                        ro = (g - 4 * q4) * 128
                                        nc.vector.tensor_mul(
                                            pt[:], pt[:],
                                            bandT[:, 384 - ro:896 - ro])
                                    nc.tensor.matmul(dn[:], onb[:], pt[:],
                                                     start=(g == 0),
                                                     stop=(g == ng - 1))
                                    nc.tensor.matmul(
                                        av[:],
                                        vsb[r][:, base + g * 128:
                                               base + (g + 1) * 128],
                                        pt[:], start=(g == 0),
                                        stop=(g == ng - 1))
                                dnr = sp.tile([1, TOK], F32, tag="dnr",
                                              bufs=2)
                                nc.vector.reciprocal(dnr[:], dn[:])
                                rdB = sp.tile([128, TOK], F32, tag="rdB",
                                              bufs=2)
                                nc.gpsimd.partition_broadcast(rdB[:], dnr[:])
                                nc.vector.tensor_mul(
                                    osb[r][:, base + q4 * TOK:
                                           base + (q4 + 1) * TOK],
                                    av[:], rdB[:])
                                cb = cbs.get((b, q4))
                                if cb is not None:
                                    cb()

                    def bounce_o(r):
                        oiv = oin_t[r].ap().rearrange("(j d) c -> j d c",
                                                      j=NCORES, d=128)
                        for j in range(NCORES):
                            nc.scalar.dma_start(
                                oiv[j], osb[r][:, j * TOK:(j + 1) * TOK])
                        nc.gpsimd.collective_compute(
                            "AllToAll", mybir.AluOpType.bypass,
                            replica_groups=RG,
                            ins=[oin_t[r].ap().opt()],
                            outs=[oout_t[r].ap().opt()])

                    def load_ao(r):
                        ov = oout_t[r].ap().rearrange("(j d) c -> j d c",
                                                      j=NCORES, d=128)
                        for j in range(NCORES):
                            nc.sync.dma_start(
                                ao[r][:, j * TOK:(j + 1) * TOK], ov[j])

                    load_round(0)
                    attn_round(0, cbs={(0, 0): bq1,
                                       (0, 3): lambda: load_round(1)})
                    bounce_o(0)
                    attn_round(1, cbs={(1, 0): lambda: load_ao(0)})
                    bounce_o(1)

                    # O-proj round 0: partial sums (even head blocks) -> oacc
                    for o in range(NF):
                        woc = m2.tile([128, 8 * 128], BF16, tag="woc",
                                      bufs=3)
                        nc.sync.dma_start(
                            woc[:].rearrange("p (s m) -> p s m", s=8),
                            wo[:, o * 128:(o + 1) * 128]
                            .rearrange("(i p) m -> p i m", p=128)[:, 0:8, :])
                        acc = ps3.tile([128, TOK], F32, tag="big")
                        for s in range(8):
                            nc.tensor.matmul(acc[:],
                                             woc[:, s * 128:(s + 1) * 128],
                                             ao[0][:, s * TOK:(s + 1) * TOK],
                                             start=(s == 0), stop=(s == 7))
                        nc.vector.tensor_copy(
                            oacc[:, o * TOK:(o + 1) * TOK], acc[:])
                        if o == 0:
                            load_ao(1)

                    # O-proj round 1: += odd head blocks, + residual, norm2
                    ssq2 = ps_dn.tile([1, TOK], F32, tag="dn", name="ssq2")
                    for o in range(NF):
                        woc = m2.tile([128, 8 * 128], BF16, tag="woc",
                                      bufs=3)
                        nc.sync.dma_start(
                            woc[:].rearrange("p (s m) -> p s m", s=8),
                            wo[:, o * 128:(o + 1) * 128]
                            .rearrange("(i p) m -> p i m", p=128)[:, 8:16, :])
                        acc = ps3.tile([128, TOK], F32, tag="big")
                        for s in range(8):
                            nc.tensor.matmul(acc[:],
                                             woc[:, s * 128:(s + 1) * 128],
                                             ao[1][:, s * TOK:(s + 1) * TOK],
                                             start=(s == 0), stop=(s == 7))
                        t2 = sp.tile([128, TOK], F32, tag="t2", bufs=2)
                        nc.vector.tensor_add(t2[:], acc[:],
                                             oacc[:, o * TOK:(o + 1) * TOK])
                        x2sl = x2[:, o * TOK:(o + 1) * TOK]
                        nc.vector.tensor_add(x2sl, t2[:],
                                             xt[:, o * TOK:(o + 1) * TOK])
                        sq = sp.tile([128, TOK], F32R, tag="sq", bufs=2)
                        nc.scalar.activation(sq[:], x2sl, AF.Square)
                        nc.tensor.matmul(ssq2[:], onr[:], sq[:],
                                         start=(o == 0), stop=(o == NF - 1))
                    rmsnorm_rs(ssq2)

            # ==================== Phase 4: norm2 + FFN ===================
            with (
                tc.tile_pool(name="m4", bufs=1) as m4,
                tc.tile_pool(name="ps4", bufs=3, space="PSUM") as ps4,
            ):
                xn2 = m4.tile([128, NF * TOK], BF16)
                for i in range(NF):
                    nc.vector.tensor_mul(xn2[:, i * TOK:(i + 1) * TOK],
                                         x2[:, i * TOK:(i + 1) * TOK],
                                         rsB[:])
                # ff1 + silu -> h (bf16, SBUF resident)
                h = m4.tile([128, 64 * TOK], BF16)
                for o in range(FF // 128):
                    wc = m4.tile([128, NF * 128], BF16, tag="wf1c", bufs=4)
                    nc.sync.dma_start(
                        wc[:].rearrange("p (i m) -> p i m", i=NF),
                        wf1[:, o * 128:(o + 1) * 128]
                        .rearrange("(i p) m -> p i m", p=128))
                    acc = ps4.tile([128, TOK], F32, tag="big")
                    for i in range(NF):
                        nc.tensor.matmul(acc[:], wc[:, i * 128:(i + 1) * 128],
                                         xn2[:, i * TOK:(i + 1) * TOK],
                                         start=(i == 0), stop=(i == NF - 1))
                    nc.scalar.activation(h[:, o * TOK:(o + 1) * TOK], acc[:],
                                         AF.Silu)
                # ff2 (bf16) + residual -> outT
                for o in range(NF):
                    wc2 = m4.tile([128, 64 * 128], BF16, tag="wf2c", bufs=2)
                    nc.sync.dma_start(
                        wc2[:].rearrange("p (k m) -> p k m", k=64),
                        wf2[:, o * 128:(o + 1) * 128]
                        .rearrange("(k p) m -> p k m", p=128))
                    acc = ps4.tile([128, TOK], F32, tag="big")
                    for k in range(64):
                        nc.tensor.matmul(acc[:],
                                         wc2[:, k * 128:(k + 1) * 128],
                                         h[:, k * TOK:(k + 1) * TOK],
                                         start=(k == 0), stop=(k == 63))
                    osl = m4.tile([128, TOK], F32, tag="osl", bufs=2)
                    nc.vector.tensor_add(osl[:], acc[:],
                                         x2[:, o * TOK:(o + 1) * TOK])
                    nc.gpsimd.dma_start(outT[o * 128:(o + 1) * 128, :],
                                        osl[:])

    nc.compile()
    return nc


_COMPILED = None


def _prep_inmaps(x, rope_cos, rope_sin, mask, w_norm1, w_norm2, wq, wk, wv,
                 wo, w_ff1, w_ff2):
    x = np.asarray(x, np.float32)
    cos = np.asarray(rope_cos, np.float32)
    sin = np.asarray(rope_sin, np.float32)
    mask = np.asarray(mask)
    wn1 = np.asarray(w_norm1, np.float32)
    wn2 = np.asarray(w_norm2, np.float32)

    bf = ml_dtypes.bfloat16
    wqn = np.ascontiguousarray(
        wn1[:, None] * np.asarray(wq, np.float32)).astype(bf)
    wkn = np.ascontiguousarray(
        wn1[:, None] * np.asarray(wk, np.float32)).astype(bf)
    wvn = np.ascontiguousarray(
        wn1[:, None] * np.asarray(wv, np.float32)).astype(bf)
    won = np.ascontiguousarray(np.asarray(wo, np.float32)).astype(bf)
    wf1n = np.ascontiguousarray(
        wn2[:, None] * np.asarray(w_ff1, np.float32)).astype(bf)
    wf2b = np.asarray(w_ff2, np.float32).astype(bf)

    # causal band mask: band[row, cc] = 1 iff cc >= row + 384 (multiplicative)
    cc = np.arange(896)[None, :]
    rr = np.arange(128)[:, None]
    band = np.where(cc >= rr + 384, 1.0, 0.0).astype(bf)
    # key-padding mask bias, [128, 2*H]: col b*16+g <- kv pos 128g+p
    mb = np.where(mask != 0, 0.0, NEG).astype(np.float32)  # [B, L]
    mbias = np.ascontiguousarray(
        mb.reshape(B, H, 128).transpose(2, 0, 1).reshape(128, B * H))

    in_maps = []
    for c in range(NCORES):
        b = c // 4
        lo = 512 * (c % 4)
        pos = slice(lo, lo + TOK)
        s = sin[pos].T.copy()
        s2 = np.concatenate([-s[:64], s[64:]], axis=0)
        in_maps.append({
            "xT": np.ascontiguousarray(x[b, pos].T),
            "wq": wqn, "wk": wkn, "wv": wvn, "wo": won,
            "wf1": wf1n, "wf2": wf2b,
            "ropeC": np.ascontiguousarray(cos[pos].T),
            "ropeS2": np.ascontiguousarray(s2),
            "band": band, "mbias": mbias,
            "onesr": np.ones((128, 1), np.float32),
            "onesb": np.ones((128, 1), bf),
        })
    return in_maps


def _assemble(res):
    out = np.empty((B, L, D), np.float32)
    for c in range(NCORES):
        b = c // 4
        lo = 512 * (c % 4)
        out[b, lo:lo + TOK, :] = res.results[c]["outT"].T
    return out


def kernel(**inputs):
    global _COMPILED
    if _COMPILED is None:
        _COMPILED = _build()
    in_maps = _prep_inmaps(**inputs)
    res = run_bass_kernel_spmd(_COMPILED, in_maps, list(range(NCORES)))
    return _assemble(res)


def timed_run(**inputs):
    """Run with NTFF profiling; returns (exec_time_ns, BassKernelResults)."""
    global _COMPILED
    if _COMPILED is None:
        _COMPILED = _build()
    in_maps = _prep_inmaps(**inputs)
    res = run_bass_kernel_spmd(_COMPILED, in_maps, list(range(NCORES)),
                               trace=True)
    return res.exec_time_ns, res


# revision 15
# speedup vs baseline: 1.0089x; 1.0089x over previous
"""Trainium2 Bass kernel for a dense transformer decoder block on 8 NeuronCores.

Sharding (uniform SPMD):
  * tokens: core c owns 512 contiguous tokens — batch c//4, positions
    [512*(c%4), 512*(c%4)+512). All projections, norms and the FFN are
    computed purely locally on those tokens.
  * attention: head-parallel via AllToAll, pipelined in two per-head rounds.
    Each core computes Q/K/V for its own tokens (all heads, feature-major,
    RoPE applied to Q/K). Round r redistributes head 2c+r: two kv AllToAlls
    (round a kicked while round b is bounced, both overlap the Q projection),
    one q AllToAll, and per-round o AllToAlls that overlap the other round's
    attention compute. Causal attention for the round's head runs fully
    on-chip; the o AllToAll routes outputs back to token owners where the
    output projection accumulates the two rounds (partial sums in SBUF).

Layout: activations are feature-major (features on SBUF partitions, tokens on
the free axis) so every matmul is transpose-free. Scores are built in S^T
orientation (kv on partitions) feeding softmax (exp on ScalarE, key-padding
mask folded into the exp bias, causal diagonal via an additive band mask)
straight into the attention*V matmul; the softmax denominator is a
ones-column matmul accumulated alongside. RMSNorm statistics use a
Square-activation + ones-matmul (cross-partition reduce on the PE).

Dtypes: bf16 operands for ALL matmuls (PSUM accumulation stays fp32), which
halves weight/A2A/DVE traffic vs fp32r at the same PE rate. The residual
path (x, x+attn) stays fp32 SBUF-resident across phases — no DRAM spill.
Norm statistics run in fp32r.

Engine assignment: sync(SP)=weight streaming, scalar(ACT)=activations +
bounce/gather DMAs, gpsimd=collectives + broadcasts + output stores.
"""
import sys

sys.path.insert(0, '/opt/trn_rl_repo')

import numpy as np
import ml_dtypes

import concourse.bacc as bacc
import concourse.mybir as mybir
from concourse import tile
from concourse.bass_utils import run_bass_kernel_spmd

F32 = mybir.dt.float32
F32R = mybir.dt.float32r
BF16 = mybir.dt.bfloat16
AF = mybir.ActivationFunctionType

D = 2048
H = 16
DH = 128
FF = 8192
B = 2
L = 2048
NCORES = 8
TOK = 512            # tokens per core
NF = D // 128        # 16 feature tiles
NEG = -30000.0
EPS = float(np.finfo(np.float32).eps)
ISQ = 1.0 / float(np.sqrt(DH))
RG = [list(range(NCORES))]


def _build():
    nc = bacc.Bacc("TRN2", target_bir_lowering=False, debug=False,
                   num_devices=NCORES)

    xT = nc.dram_tensor("xT", [D, TOK], F32, kind="ExternalInput")
    wq = nc.dram_tensor("wq", [D, D], BF16, kind="ExternalInput")
    wk = nc.dram_tensor("wk", [D, D], BF16, kind="ExternalInput")
    wv = nc.dram_tensor("wv", [D, D], BF16, kind="ExternalInput")
    wo = nc.dram_tensor("wo", [D, D], BF16, kind="ExternalInput")
    wf1 = nc.dram_tensor("wf1", [D, FF], BF16, kind="ExternalInput")
    wf2 = nc.dram_tensor("wf2", [FF, D], BF16, kind="ExternalInput")
    ropeC = nc.dram_tensor("ropeC", [DH, TOK], F32, kind="ExternalInput")
    ropeS2 = nc.dram_tensor("ropeS2", [DH, TOK], F32, kind="ExternalInput")
    band = nc.dram_tensor("band", [128, 896], BF16, kind="ExternalInput")
    mbias = nc.dram_tensor("mbias", [128, 2 * H], F32, kind="ExternalInput")
    onesr = nc.dram_tensor("onesr", [128, 1], F32R, kind="ExternalInput")
    onesb = nc.dram_tensor("onesb", [128, 1], BF16, kind="ExternalInput")
    outT = nc.dram_tensor("outT", [D, TOK], F32, kind="ExternalOutput")

    # internal DRAM AllToAll bounce buffers (bf16)
    kvinA = nc.dram_tensor("kvinA", [NCORES * 256, TOK], BF16)
    kvinB = nc.dram_tensor("kvinB", [NCORES * 256, TOK], BF16)
    kvoutA = nc.dram_tensor("kvoutA", [NCORES * 256, TOK], BF16)
    kvoutB = nc.dram_tensor("kvoutB", [NCORES * 256, TOK], BF16)
    qinA = nc.dram_tensor("qinA", [NCORES * 128, TOK], BF16)
    qinB = nc.dram_tensor("qinB", [NCORES * 128, TOK], BF16)
    qoutA = nc.dram_tensor("qoutA", [NCORES * 128, TOK], BF16)
    qoutB = nc.dram_tensor("qoutB", [NCORES * 128, TOK], BF16)
    oinA = nc.dram_tensor("oinA", [NCORES * 128, TOK], BF16)
    oinB = nc.dram_tensor("oinB", [NCORES * 128, TOK], BF16)
    ooutA = nc.dram_tensor("ooutA", [NCORES * 128, TOK], BF16)
    ooutB = nc.dram_tensor("ooutB", [NCORES * 128, TOK], BF16)

    with tile.TileContext(nc) as tc:
        with (
            tc.tile_pool(name="const", bufs=1) as cp,
            tc.tile_pool(name="small", bufs=1) as sp,
        ):
            cosT = cp.tile([DH, TOK], F32)
            sin2 = cp.tile([DH, TOK], F32)
            bandT = cp.tile([128, 896], BF16)
            mbT = cp.tile([128, 2 * H], F32)
            onr = cp.tile([128, 1], F32R)
            onb = cp.tile([128, 1], BF16)
            epsc = cp.tile([1, 1], F32)
            x2 = cp.tile([128, NF * TOK], F32)      # resident x + attn@wo
            nc.scalar.dma_start(cosT[:], ropeC[:])
            nc.scalar.dma_start(sin2[:], ropeS2[:])
            nc.scalar.dma_start(bandT[:], band[:])
            nc.scalar.dma_start(mbT[:], mbias[:])
            nc.scalar.dma_start(onr[:], onesr[:])
            nc.scalar.dma_start(onb[:], onesb[:])
            nc.gpsimd.memset(epsc[:], EPS)

            rsB = sp.tile([128, TOK], F32)
            rowS = sp.tile([1, TOK], F32)
            rowR = sp.tile([1, TOK], F32)

            def rmsnorm_rs(ssq_ps):
                nc.scalar.activation(rowS[:], ssq_ps[:], AF.Sqrt,
                                     bias=epsc[:], scale=1.0 / D)
                nc.vector.reciprocal(rowR[:], rowS[:])
                nc.gpsimd.partition_broadcast(rsB[:], rowR[:])

            with tc.tile_pool(name="mx", bufs=1) as mxp:
                xt = mxp.tile([128, NF * TOK], F32)   # resident input x^T
                for qq in range(4):
                    nc.scalar.dma_start(
                        xt[:, qq * 4 * TOK:(qq + 1) * 4 * TOK]
                        .rearrange("p (i c) -> p i c", i=4),
                        xT[qq * 512:(qq + 1) * 512, :]
                        .rearrange("(i p) c -> p i c", p=128))

                # ========== Phase 1: norm1, K^T, V, Q^T, AllToAlls ==========
                with (
                    tc.tile_pool(name="m1", bufs=1) as m1,
                    tc.tile_pool(name="ps1", bufs=3, space="PSUM") as ps1,
                    tc.tile_pool(name="psr", bufs=1, space="PSUM") as psr,
                ):
                    ssq = psr.tile([1, TOK], F32, tag="row")
                    for i in range(NF):
                        sq = sp.tile([128, TOK], F32R, tag="sq", bufs=2)
                        nc.scalar.activation(sq[:],
                                             xt[:, i * TOK:(i + 1) * TOK],
                                             AF.Square)
                        nc.tensor.matmul(ssq[:], onr[:], sq[:],
                                         start=(i == 0), stop=(i == NF - 1))
                    rmsnorm_rs(ssq)
                    xnt = m1.tile([128, NF * TOK], BF16, tag="xn")
                    for i in range(NF):
                        nc.vector.tensor_mul(xnt[:, i * TOK:(i + 1) * TOK],
                                             xt[:, i * TOK:(i + 1) * TOK],
                                             rsB[:])

                    def proj_T(wten, out_tile, tag, o_range):
                        """out_tile[:, o*TOK:] = head-tile o of (xn @ w)^T,
                        roped."""
                        for o in o_range:
                            wc = m1.tile([128, NF * 128], BF16, tag=tag,
                                         bufs=3)
                            nc.sync.dma_start(
                                wc[:].rearrange("p (i m) -> p i m", i=NF),
                                wten[:, o * 128:(o + 1) * 128]
                                .rearrange("(i p) m -> p i m", p=128))
                            acc = ps1.tile([128, TOK], F32, tag="big")
                            for i in range(NF):
                                nc.tensor.matmul(
                                    acc[:], wc[:, i * 128:(i + 1) * 128],
                                    xnt[:, i * TOK:(i + 1) * TOK],
                                    start=(i == 0), stop=(i == NF - 1))
                            dst = out_tile[:, o * TOK:(o + 1) * TOK]
                            tmp = sp.tile([128, TOK], BF16, tag="rtmp",
                                          bufs=2)
                            nc.vector.tensor_mul(tmp[0:64, :], acc[64:128, :],
                                                 sin2[0:64, :])
                            nc.vector.tensor_mul(tmp[64:128, :], acc[0:64, :],
                                                 sin2[64:128, :])
                            nc.vector.tensor_mul(dst, acc[:], cosT[:])
                            nc.vector.tensor_add(dst, dst, tmp[:])

                    kt = m1.tile([128, NF * TOK], BF16, tag="kt")
                    vt = m1.tile([128, 4 * D], BF16, tag="vt")
                    qt = m1.tile([128, NF * TOK], BF16, tag="qt")
                    ktv = kt[:].rearrange("p (o c) -> p o c", o=NF)
                    vtv = vt[:].rearrange("p (t f) -> p t f", t=4)
                    qtv = qt[:].rearrange("p (o c) -> p o c", o=NF)

                    def v_half(cc_range):
                        for cc in cc_range:
                            wvc = m1.tile([128, NF * TOK], BF16, tag="wvc",
                                          bufs=2, name="wvc")
                            nc.sync.dma_start(
                                wvc[:].rearrange("p (i m) -> p i m", i=NF),
                                wv[:, cc * TOK:(cc + 1) * TOK]
                                .rearrange("(i p) m -> p i m", p=128))
                            for to in range(4):
                                acc = ps1.tile([128, TOK], F32, tag="big",
                                               name="acc")
                                for i in range(NF):
                                    nc.tensor.matmul(
                                        acc[:],
                                        xnt[:, i * TOK + to * 128:
                                            i * TOK + (to + 1) * 128],
                                        wvc[:, i * TOK:(i + 1) * TOK],
                                        start=(i == 0), stop=(i == NF - 1))
                                nc.vector.tensor_copy(
                                    vt[:, to * D + cc * TOK:
                                       to * D + (cc + 1) * TOK], acc[:])

                    def bounce_kv(r, kvin_t, kvout_t):
                        # round r carries head j+8r for dest core j
                        kviv = kvin_t.ap().rearrange(
                            "(j s d) (t f) -> j s d t f", j=NCORES, s=2,
                            d=128, t=4, f=128)
                        for j in range(NCORES):
                            hh = j + 8 * r
                            nc.scalar.dma_start(
                                kviv[j, 0].rearrange("d t f -> d (t f)"),
                                ktv[:, hh, :])
                            nc.scalar.dma_start(
                                kviv[j, 1],
                                vtv[:, :, hh * 128:(hh + 1) * 128])
                        nc.gpsimd.collective_compute(
                            "AllToAll", mybir.AluOpType.bypass,
                            replica_groups=RG,
                            ins=[kvin_t.ap().opt()],
                            outs=[kvout_t.ap().opt()])

                    def bounce_q(r, qin_t, qout_t):
                        qiv = qin_t.ap().rearrange("(j d) c -> j d c",
                                                   j=NCORES, d=128)
                        for j in range(NCORES):
                            nc.scalar.dma_start(qiv[j], qtv[:, j + 8 * r, :])
                        nc.gpsimd.collective_compute(
                            "AllToAll", mybir.AluOpType.bypass,
                            replica_groups=RG,
                            ins=[qin_t.ap().opt()], outs=[qout_t.ap().opt()])

                    # 4-deep pipeline: each half ships while the next computes
                    proj_T(wk, kt, "wc", range(0, 8))
                    v_half(range(0, 2))
                    bounce_kv(0, kvinA, kvoutA)
                    proj_T(wk, kt, "wc", range(8, 16))
                    v_half(range(2, 4))
                    bounce_kv(1, kvinB, kvoutB)
                    proj_T(wq, qt, "wc", range(0, 8))
                    bounce_q(0, qinA, qoutA)
                    proj_T(wq, qt, "wc", range(8, 16))
                    bq1 = lambda: bounce_q(1, qinB, qoutB)

                # ====== Phase 2+3: attention rounds + O-projection ======
                with (
                    tc.tile_pool(name="m2", bufs=1) as m2,
                    tc.tile_pool(name="ps_s", bufs=2, space="PSUM") as ps_s,
                    tc.tile_pool(name="ps_av", bufs=2, space="PSUM") as ps_av,
                    tc.tile_pool(name="ps_dn", bufs=2, space="PSUM") as ps_dn,
                    tc.tile_pool(name="ps3", bufs=2, space="PSUM") as ps3,
                ):
                    ksb = [m2.tile([128, 8 * TOK], BF16, tag=f"ksb{r}",
                                   name=f"ksb{r}") for r in range(2)]
                    vsb = [m2.tile([128, 8 * TOK], BF16, tag=f"vsb{r}",
                                   name=f"vsb{r}") for r in range(2)]
                    qsb = [m2.tile([128, 8 * TOK], BF16, tag=f"qsb{r}",
                                   name=f"qsb{r}") for r in range(2)]
                    osb = [m2.tile([128, 8 * TOK], BF16, tag=f"osb{r}",
                                   name=f"osb{r}") for r in range(2)]
                    ao = [m2.tile([128, 8 * TOK], BF16, tag=f"ao{r}",
                                  name=f"ao{r}") for r in range(2)]
                    oacc = m2.tile([128, NF * TOK], BF16, tag="oacc")

                    kvout_t = (kvoutA, kvoutB)
                    qout_t = (qoutA, qoutB)
                    oin_t = (oinA, oinB)
                    oout_t = (ooutA, ooutB)

                    def load_round(r):
                        eng = nc.scalar if r == 0 else nc.sync
                        kvv = kvout_t[r].ap().rearrange(
                            "(j s d) c -> j s d c", j=NCORES, s=2, d=128)
                        qov = qout_t[r].ap().rearrange(
                            "(j d) c -> j d c", j=NCORES, d=128)
                        for j in range(NCORES):
                            eng.dma_start(
                                ksb[r][:, j * TOK:(j + 1) * TOK], kvv[j, 0])
                            eng.dma_start(
                                vsb[r][:, j * TOK:(j + 1) * TOK], kvv[j, 1])
                            eng.dma_start(
                                qsb[r][:, j * TOK:(j + 1) * TOK], qov[j])

                    def attn_round(r, cbs={}):
                        for b in range(B):
                            for q4 in range(4):
                                base = b * 4 * TOK
                                qs = qsb[r][:, base + q4 * TOK:
                                            base + (q4 + 1) * TOK]
                                ng = 4 * q4 + 4
                                av = ps_av.tile([128, TOK], F32, tag="av")
                                dn = ps_dn.tile([1, TOK], F32, tag="dn")
                                for g in range(ng):
                                    st = ps_s.tile([128, TOK], F32, tag="s")
                                    nc.tensor.matmul(
                                        st[:],
                                        ksb[r][:, base + g * 128:
                                               base + (g + 1) * 128],
                                        qs, start=True, stop=True)
                                    pt = sp.tile([128, TOK], BF16, tag="pt",
                                                 bufs=3)
                                    nc.scalar.activation(
                                        pt[:], st[:], AF.Exp,
                                        bias=mbT[:, b * H + g:b * H + g + 1],
                                        scale=ISQ)
                                    if g >= 4 * q4:
                                        ro = (g - 4 * q4) * 128
                                        nc.vector.tensor_mul(
                                            pt[:], pt[:],
                                            bandT[:, 384 - ro:896 - ro])
                                    nc.tensor.matmul(dn[:], onb[:], pt[:],
                                                     start=(g == 0),
                                                     stop=(g == ng - 1))
                                    nc.tensor.matmul(
                                        av[:],
                                        vsb[r][:, base + g * 128:
                                               base + (g + 1) * 128],
                                        pt[:], start=(g == 0),
                                        stop=(g == ng - 1))
                                dnr = sp.tile([1, TOK], F32, tag="dnr",
                                              bufs=2)
                                nc.vector.reciprocal(dnr[:], dn[:])
                                rdB = sp.tile([128, TOK], F32, tag="rdB",
                                              bufs=2)
                                nc.gpsimd.partition_broadcast(rdB[:], dnr[:])
                                nc.vector.tensor_mul(
                                    osb[r][:, base + q4 * TOK:
                                           base + (q4 + 1) * TOK],
                                    av[:], rdB[:])
                                cb = cbs.get((b, q4))
                                if cb is not None:
                                    cb()

                    def bounce_o(r):
                        oiv = oin_t[r].ap().rearrange("(j d) c -> j d c",
                                                      j=NCORES, d=128)
                        for j in range(NCORES):
                            nc.scalar.dma_start(
                                oiv[j], osb[r][:, j * TOK:(j + 1) * TOK])
                        nc.gpsimd.collective_compute(
                            "AllToAll", mybir.AluOpType.bypass,
                            replica_groups=RG,
                            ins=[oin_t[r].ap().opt()],
                            outs=[oout_t[r].ap().opt()])

                    def load_ao(r):
                        ov = oout_t[r].ap().rearrange("(j d) c -> j d c",
                                                      j=NCORES, d=128)
                        for j in range(NCORES):
                            nc.sync.dma_start(
                                ao[r][:, j * TOK:(j + 1) * TOK], ov[j])

                    load_round(0)
                    attn_round(0, cbs={(0, 0): bq1,
                                       (0, 3): lambda: load_round(1)})
                    bounce_o(0)
                    attn_round(1, cbs={(1, 0): lambda: load_ao(0)})
                    bounce_o(1)

                    # O-proj round 0: partial sums (even head blocks) -> oacc
                    for o in range(NF):
                        woc = m2.tile([128, 8 * 128], BF16, tag="woc",
                                      bufs=3)
                        nc.sync.dma_start(
                            woc[:].rearrange("p (s m) -> p s m", s=8),
                            wo[:, o * 128:(o + 1) * 128]
                            .rearrange("(i p) m -> p i m", p=128)[:, 0:8, :])
                        acc = ps3.tile([128, TOK], F32, tag="big")
                        for s in range(8):
                            nc.tensor.matmul(acc[:],
                                             woc[:, s * 128:(s + 1) * 128],
                                             ao[0][:, s * TOK:(s + 1) * TOK],
                                             start=(s == 0), stop=(s == 7))
                        nc.vector.tensor_copy(
                            oacc[:, o * TOK:(o + 1) * TOK], acc[:])
                        if o == 0:
                            load_ao(1)

                    # O-proj round 1: += odd head blocks, + residual, norm2
                    ssq2 = ps_dn.tile([1, TOK], F32, tag="dn", name="ssq2")
                    for o in range(NF):
                        woc = m2.tile([128, 8 * 128], BF16, tag="woc",
                                      bufs=3)
                        nc.sync.dma_start(
                            woc[:].rearrange("p (s m) -> p s m", s=8),
                            wo[:, o * 128:(o + 1) * 128]
                            .rearrange("(i p) m -> p i m", p=128)[:, 8:16, :])
                        acc = ps3.tile([128, TOK], F32, tag="big")
                        for s in range(8):
                            nc.tensor.matmul(acc[:],
                                             woc[:, s * 128:(s + 1) * 128],
                                             ao[1][:, s * TOK:(s + 1) * TOK],
                                             start=(s == 0), stop=(s == 7))
                        t2 = sp.tile([128, TOK], F32, tag="t2", bufs=2)
                        nc.vector.tensor_add(t2[:], acc[:],
                                             oacc[:, o * TOK:(o + 1) * TOK])
                        x2sl = x2[:, o * TOK:(o + 1) * TOK]
                        nc.vector.tensor_add(x2sl, t2[:],
                                             xt[:, o * TOK:(o + 1) * TOK])
                        sq = sp.tile([128, TOK], F32R, tag="sq", bufs=2)
                        nc.scalar.activation(sq[:], x2sl, AF.Square)
                        nc.tensor.matmul(ssq2[:], onr[:], sq[:],
                                         start=(o == 0), stop=(o == NF - 1))
                    rmsnorm_rs(ssq2)

            # ==================== Phase 4: norm2 + FFN ===================
            with (
                tc.tile_pool(name="m4", bufs=1) as m4,
                tc.tile_pool(name="ps4", bufs=3, space="PSUM") as ps4,
            ):
                xn2 = m4.tile([128, NF * TOK], BF16)
                for i in range(NF):
                    nc.vector.tensor_mul(xn2[:, i * TOK:(i + 1) * TOK],
                                         x2[:, i * TOK:(i + 1) * TOK],
                                         rsB[:])
                # ff1 + silu -> h (bf16, SBUF resident)
                h = m4.tile([128, 64 * TOK], BF16)
                for o in range(FF // 128):
                    wc = m4.tile([128, NF * 128], BF16, tag="wf1c", bufs=3)
                    nc.sync.dma_start(
                        wc[:].rearrange("p (i m) -> p i m", i=NF),
                        wf1[:, o * 128:(o + 1) * 128]
                        .rearrange("(i p) m -> p i m", p=128))
                    acc = ps4.tile([128, TOK], F32, tag="big")
                    for i in range(NF):
                        nc.tensor.matmul(acc[:], wc[:, i * 128:(i + 1) * 128],
                                         xn2[:, i * TOK:(i + 1) * TOK],
                                         start=(i == 0), stop=(i == NF - 1))
                    nc.scalar.activation(h[:, o * TOK:(o + 1) * TOK], acc[:],
                                         AF.Silu)
                # ff2 (bf16) + residual -> outT
                for o in range(NF):
                    wc2 = m4.tile([128, 64 * 128], BF16, tag="wf2c", bufs=2)
                    nc.sync.dma_start(
                        wc2[:].rearrange("p (k m) -> p k m", k=64),
                        wf2[:, o * 128:(o + 1) * 128]
                        .rearrange("(k p) m -> p k m", p=128))
                    acc = ps4.tile([128, TOK], F32, tag="big")
                    for k in range(64):
                        nc.tensor.matmul(acc[:],
                                         wc2[:, k * 128:(k + 1) * 128],
                                         h[:, k * TOK:(k + 1) * TOK],
                                         start=(k == 0), stop=(k == 63))
                    osl = m4.tile([128, TOK], F32, tag="osl", bufs=2)
                    nc.vector.tensor_add(osl[:], acc[:],
                                         x2[:, o * TOK:(o + 1) * TOK])
                    nc.gpsimd.dma_start(outT[o * 128:(o + 1) * 128, :],
                                        osl[:])

    nc.compile()
    return nc


_COMPILED = None


def _prep_inmaps(x, rope_cos, rope_sin, mask, w_norm1, w_norm2, wq, wk, wv,
                 wo, w_ff1, w_ff2):
    x = np.asarray(x, np.float32)
    cos = np.asarray(rope_cos, np.float32)
    sin = np.asarray(rope_sin, np.float32)
    mask = np.asarray(mask)
    wn1 = np.asarray(w_norm1, np.float32)
    wn2 = np.asarray(w_norm2, np.float32)

    bf = ml_dtypes.bfloat16
    wqn = np.ascontiguousarray(
        wn1[:, None] * np.asarray(wq, np.float32)).astype(bf)
    wkn = np.ascontiguousarray(
        wn1[:, None] * np.asarray(wk, np.float32)).astype(bf)
    wvn = np.ascontiguousarray(
        wn1[:, None] * np.asarray(wv, np.float32)).astype(bf)
    won = np.ascontiguousarray(np.asarray(wo, np.float32)).astype(bf)
    wf1n = np.ascontiguousarray(
        wn2[:, None] * np.asarray(w_ff1, np.float32)).astype(bf)
    wf2b = np.asarray(w_ff2, np.float32).astype(bf)

    # causal band mask: band[row, cc] = 1 iff cc >= row + 384 (multiplicative)
    cc = np.arange(896)[None, :]
    rr = np.arange(128)[:, None]
    band = np.where(cc >= rr + 384, 1.0, 0.0).astype(bf)
    # key-padding mask bias, [128, 2*H]: col b*16+g <- kv pos 128g+p
    mb = np.where(mask != 0, 0.0, NEG).astype(np.float32)  # [B, L]
    mbias = np.ascontiguousarray(
        mb.reshape(B, H, 128).transpose(2, 0, 1).reshape(128, B * H))

    in_maps = []
    for c in range(NCORES):
        b = c // 4
        lo = 512 * (c % 4)
        pos = slice(lo, lo + TOK)
        s = sin[pos].T.copy()
        s2 = np.concatenate([-s[:64], s[64:]], axis=0)
        in_maps.append({
            "xT": np.ascontiguousarray(x[b, pos].T),
            "wq": wqn, "wk": wkn, "wv": wvn, "wo": won,
            "wf1": wf1n, "wf2": wf2b,
            "ropeC": np.ascontiguousarray(cos[pos].T),
            "ropeS2": np.ascontiguousarray(s2),
            "band": band, "mbias": mbias,
            "onesr": np.ones((128, 1), np.float32),
            "onesb": np.ones((128, 1), bf),
        })
    return in_maps


def _assemble(res):
    out = np.empty((B, L, D), np.float32)
    for c in range(NCORES):
        b = c // 4
        lo = 512 * (c % 4)
        out[b, lo:lo + TOK, :] = res.results[c]["outT"].T
    return out


def kernel(**inputs):
    global _COMPILED
    if _COMPILED is None:
        _COMPILED = _build()
    in_maps = _prep_inmaps(**inputs)
    res = run_bass_kernel_spmd(_COMPILED, in_maps, list(range(NCORES)))
    return _assemble(res)


def timed_run(**inputs):
    """Run with NTFF profiling; returns (exec_time_ns, BassKernelResults)."""
    global _COMPILED
    if _COMPILED is None:
        _COMPILED = _build()
    in_maps = _prep_inmaps(**inputs)
    res = run_bass_kernel_spmd(_COMPILED, in_maps, list(range(NCORES)),
                               trace=True)
    return res.exec_time_ns, res
